# revision 46
# baseline (speedup 1.0000x reference)
"""Trainium2 Bass kernel for PVT-style spatial-reduction attention.

Model (see reference):
  q = (x @ Wq + bq) * hd^-0.5                       (B, N, C) -> heads of 32
  x_ = BN(DWConv2x2s2(x)) ; k = x_ @ Wk ; v = x_ @ Wv
  attn = softmax(q k^T + rel_pos) ; out = (attn @ v) @ Wp + bp

Shapes: B=8, N=3136 (56x56), C=128, heads=4, hd=32, Nkv=784 (28x28).

Distribution: each of 8 cores handles a slice of 392 query rows (N/8) for
ALL batches and heads; rel_pos splits exactly 8 ways; no collectives.

Device layout strategy (v2):
  - host folds conv+BN (exact fp32) producing x_^T (B, C, 784) bf16; k-bias
    dropped (softmax-invariant), v-bias folded into the final bias (added on
    host during the gather).
  - kv chunking is 7 rounds x 112 (784 = 7*112 exactly).
  - scores are computed transposed S^T[m, n] per (b, h) in half-rounds of 2
    heads on concurrent 32-row PE tiles -> [C, 2, 512] f32 PSUM (2 banks,
    2 bufs); one ScalarE exp per half-round (FD=784); softmax uses
    exp(S)*exp(R) with exp(rel^T) bf16 precomputed host side; ScalarE runs
    NOTHING but exp.
  - v is produced directly in [m, d] layout (lhsT = x_^T chunk streaming
    Wv), all chunks through a single rotating PSUM bank -> no PE
    transposes at all.
  - attn@v: per round 4 col-packed matmuls (head h -> bank 0, output
    partitions 32h..32h+31) + 4 col-packed ones-lhsT [112, 32] matmuls
    that write each head's row sums REPLICATED across the same partition
    block of bank 1, accumulating f32 over the 7 rounds onto PE-zeroed
    banks.  The attnv output lands directly in outT [d, n] partition
    order and the row sums are lane-aligned with it, so normalization
    is one reciprocal + one multiply per batch, with the projection
    reusing the freed attnv bank.
  - final output is produced transposed (B, C, NSL); the host gather
    untransposes while assembling the full (B, N, C) result and adds the
    folded output bias.
"""

import sys

import numpy as np

if "/opt/trn_rl_repo" not in sys.path:
    sys.path.insert(0, "/opt/trn_rl_repo")

B = 8
N = 3136
C = 128
HEADS = 4
HD = 32
SR = 2
H = W = 56
NKV = 784  # 28*28
NCORES = 8
NSL = N // NCORES  # 392 query rows per core
BN_EPS = 1e-5
SCALE = HD ** -0.5

MC = 112            # kv chunk size
NR = NKV // MC      # 7 rounds

_COMPILED = None  # cached nc across kernel() calls


def _host_prep(x, relative_pos, Wq, bq, Wk, bk, Wv, bv, conv_w, conv_b,
               bn_gamma, bn_beta, bn_mean, bn_var, Wp, bp):
    """Fold conv/BN on host (exact fp32); fold biases; transpose activations."""
    import ml_dtypes
    f32 = np.float32
    bf16 = ml_dtypes.bfloat16
    x = np.asarray(x, f32)
    # xT: (B, C, N) for the q projection (each core slices its own 392 cols)
    xT = np.ascontiguousarray(x.transpose(0, 2, 1)).astype(bf16)

    # conv + BN (exact, running stats)
    inv = (np.asarray(bn_gamma, f32)
           / np.sqrt(np.asarray(bn_var, f32) + BN_EPS))          # [c]
    cw = np.asarray(conv_w, f32).reshape(C, SR, SR)
    x_img = x.transpose(0, 2, 1).reshape(B, C, H, W)
    y = np.zeros((B, C, H // SR, W // SR), f32)
    for di in range(SR):
        for dj in range(SR):
            y += x_img[:, :, di::SR, dj::SR] * cw[None, :, di, dj, None, None]
    y += np.asarray(conv_b, f32)[None, :, None, None]
    y = y * inv[None, :, None, None] \
        + (np.asarray(bn_beta, f32)
           - np.asarray(bn_mean, f32) * inv)[None, :, None, None]
    xkT = np.ascontiguousarray(y.reshape(B, C, NKV)).astype(bf16)  # (B, C, 784)

    # v bias (uniform over kv positions -> exact fold into final bias)
    bp_host = (np.asarray(bp, np.float64)
               + np.asarray(bv, np.float64) @ np.asarray(Wp, np.float64))
    bp_host = bp_host.astype(f32)

    Wq_s = np.ascontiguousarray((np.asarray(Wq, f32) * SCALE)).astype(bf16)
    bq_col = (np.asarray(bq, f32) * SCALE).reshape(C, 1).astype(f32)
    Wk_b = np.ascontiguousarray(np.asarray(Wk, f32)).astype(bf16)
    Wv_b = np.ascontiguousarray(np.asarray(Wv, f32)).astype(bf16)
    Wp_b = np.ascontiguousarray(np.asarray(Wp, f32)).astype(bf16)

    # exp(rel)^T per core: (4, NKV, NSL) bf16
    rel = np.asarray(relative_pos, f32)
    expRT = []
    for j in range(NCORES):
        sl = rel[:, j * NSL:(j + 1) * NSL, :]          # (4, NSL, NKV)
        e = np.exp(sl).transpose(0, 2, 1).astype(bf16)  # (4, NKV, NSL)
        expRT.append(np.ascontiguousarray(e))

    return dict(xT=xT, xkT=xkT, Wq=Wq_s, bq=bq_col, Wk=Wk_b, Wv=Wv_b,
                Wp=Wp_b, bp=bp_host, expRT=expRT)


def _build():
    """Build + compile the SPMD bass program (same NEFF for all 8 cores)."""
    import concourse.bass as bass
    import concourse.tile as tile
    from concourse import bacc, mybir

    f32 = mybir.dt.float32
    f32r = mybir.dt.float32r
    bf16 = mybir.dt.bfloat16

    nc = bacc.Bacc("TRN2", target_bir_lowering=False, debug=False,
                   num_devices=NCORES)

    # ---- DRAM I/O ----
    xTn_d = nc.dram_tensor("xTn", [B, C, NSL], bf16, kind="ExternalInput").ap()
    xkT_d = nc.dram_tensor("xkT", [B, C, NKV], bf16, kind="ExternalInput").ap()
    expRT_d = nc.dram_tensor("expRT", [HEADS, NKV, NSL],
                             bf16, kind="ExternalInput").ap()
    Wq_d = nc.dram_tensor("Wq", [C, C], bf16, kind="ExternalInput").ap()
    bq_d = nc.dram_tensor("bq", [C, 1], f32, kind="ExternalInput").ap()
    Wk_d = nc.dram_tensor("Wk", [C, C], bf16, kind="ExternalInput").ap()
    Wv_d = nc.dram_tensor("Wv", [C, C], bf16, kind="ExternalInput").ap()
    Wp_d = nc.dram_tensor("Wp", [C, C], bf16, kind="ExternalInput").ap()
    out_d = nc.dram_tensor("out", [B, C, NSL], f32, kind="ExternalOutput").ap()

    with tile.TileContext(nc) as tc:
        from contextlib import ExitStack
        with ExitStack() as ctx:
            _emit(ctx, tc, nc, bass, mybir, f32, f32r, bf16,
                  xTn_d, xkT_d, expRT_d, Wq_d, bq_d, Wk_d, Wv_d, Wp_d,
                  out_d)

    nc.compile()
    return nc


def _emit(ctx, tc, nc, bass, mybir, f32, f32r, bf16,
          xTn_d, xkT_d, expRT_d, Wq_d, bq_d, Wk_d, Wv_d, Wp_d,
          out_d):
    AF = mybir.ActivationFunctionType

    singles = ctx.enter_context(tc.tile_pool(name="singles", bufs=1))
    xpool = ctx.enter_context(tc.tile_pool(name="xpool", bufs=3))
    qkv = ctx.enter_context(tc.tile_pool(name="qkv", bufs=3))
    ptpool = ctx.enter_context(tc.tile_pool(name="ptpool", bufs=4))
    ppool = ctx.enter_context(tc.tile_pool(name="ppool", bufs=2))
    opool = ctx.enter_context(tc.tile_pool(name="opool", bufs=3))
    # PSUM (8 banks): sco 3 tiles x 2 banks (also borrowed by the small
    # prep/proj matmuls; 3-deep so score matmuls never head-block the PE
    # queue waiting on the exp ping-pong), ov 1 tile x 2 banks.
    ps_sco = ctx.enter_context(tc.tile_pool(name="ps_sco", bufs=3,
                                            space="PSUM"))
    ps_ov = ctx.enter_context(tc.tile_pool(name="ps_ov", bufs=1, space="PSUM"))

    # ---- constants ----
    wq_sb = singles.tile([C, C], bf16)
    nc.sync.dma_start(out=wq_sb[:], in_=Wq_d)
    bq_sb = singles.tile([C, 1], f32)
    nc.sync.dma_start(out=bq_sb[:], in_=bq_d)
    wk_sb = singles.tile([C, C], bf16)
    nc.sync.dma_start(out=wk_sb[:], in_=Wk_d)
    wv_sb = singles.tile([C, C], bf16)
    nc.sync.dma_start(out=wv_sb[:], in_=Wv_d)
    wp_sb = singles.tile([C, C], bf16)
    nc.sync.dma_start(out=wp_sb[:], in_=Wp_d)
    ones_sb = singles.tile([C, HD], bf16)
    nc.vector.memset(ones_sb[:], 1.0)
    zeros_sb = singles.tile([C, C], bf16)
    nc.vector.memset(zeros_sb[:], 0.0)

    # expRT interleaved: [112, 7 rounds, 4 heads, 392].  Loaded lazily
    # (emitted in the schedule after the batch-0 input loads) as per-round
    # contiguous-source DMAs so round 0 lands fast and the rest streams
    # behind compute instead of delaying the first scores.
    expTI = singles.tile([C, NR, HEADS, NSL], bf16)

    def load_expTI():
        for r in range(NR):
            for h in range(HEADS):
                nc.sync.dma_start(
                    out=expTI[0:MC, r, h, :],
                    in_=expRT_d[h, MC * r:MC * (r + 1), :])

    state = {}

    def prep_load(b):
        s = state.setdefault(b, {})
        xkT_sb = xpool.tile([C, NKV], bf16, tag="xkT")
        s["xkT"] = xkT_sb
        nc.sync.dma_start(out=xkT_sb[:], in_=xkT_d[b])
        xTn_sb = xpool.tile([C, NSL], bf16, tag="xTn")
        s["xTn"] = xTn_sb
        nc.sync.dma_start(out=xTn_sb[:], in_=xTn_d[b])

    def prep_q(b):
        s = state[b]
        ps_q = ps_sco.tile([C, 2, 512], f32, tag="sco")
        nc.tensor.matmul(ps_q[:, 0, 0:NSL], lhsT=wq_sb[:], rhs=s.pop("xTn")[:],
                         start=True, stop=True)
        qT_sb = qkv.tile([C, NSL], bf16, tag="qT")
        s["qT"] = qT_sb
        nc.vector.tensor_scalar_add(qT_sb[:], ps_q[:, 0, 0:NSL], bq_sb[:, 0:1])

    def prep_k(b, half):
        s = state[b]
        if half == 0:
            kT_sb = qkv.tile([C, NKV], bf16, tag="kT")
            s["kT"] = kT_sb
        ps_k = ps_sco.tile([C, 2, 512], f32, tag="sco")
        nc.tensor.matmul(ps_k[:, 0, 0:NSL], lhsT=wk_sb[:],
                         rhs=s["xkT"][:, half * NSL:(half + 1) * NSL],
                         start=True, stop=True)
        nc.vector.tensor_copy(s["kT"][:, half * NSL:(half + 1) * NSL],
                              ps_k[:, 0, 0:NSL])

    def prep_v(b, part):
        """v chunks in [m, d] layout: lhsT = xkT chunk, rhs = Wv."""
        s = state[b]
        if part == 0:
            v_sb = qkv.tile([C, NR, HEADS, HD], bf16, tag="v")
            s["v"] = v_sb
            rr = range(0, 4)
        else:
            rr = range(4, NR)
        ps_v = ps_sco.tile([C, 2, 512], f32, tag="sco")
        pv = ps_v[:].rearrange("p j (a d) -> p (j a) d", d=C)
        for i, r in enumerate(rr):
            nc.tensor.matmul(pv[0:MC, i, :],
                             lhsT=s["xkT"][:, MC * r:MC * (r + 1)],
                             rhs=wv_sb[:], start=True, stop=True)
        nv = len(rr)
        nc.vector.tensor_copy(
            s["v"][0:MC, 4 * part:4 * part + nv]
                .rearrange("p r h d -> p (r h d)"),
            pv[0:MC, 0:nv, :].rearrange("p a d -> p (a d)"))
        if part == 1:
            s.pop("xkT")

    def score_round(b, r):
        """All 4 head score matmuls back-to-back (4-way row-strip
        concurrency across two 2-bank tiles), then exp + expR multiply
        per head pair."""
        s = state[b]
        tiles = (ps_sco.tile([C, 2, 512], f32, tag="sco", name="ps_sA"),
                 ps_sco.tile([C, 2, 512], f32, tag="sco", name="ps_sB"))
        for h in range(HEADS):
            nc.tensor.matmul(
                tiles[h // 2][0:MC, h % 2, 0:NSL],
                lhsT=s["kT"][HD * h:HD * (h + 1), MC * r:MC * (r + 1)],
                rhs=s["qT"][HD * h:HD * (h + 1), :],
                start=True, stop=True,
                tile_position=(HD * h, 0))
        for hp, t in enumerate(tiles):
            pt_sb = ptpool.tile([C, 2, NSL], bf16, tag="pt")
            nc.scalar.activation(pt_sb[0:MC], t[0:MC, :, 0:NSL], AF.Exp)
            nc.vector.tensor_mul(s["pp"][0:MC, r, 2 * hp:2 * hp + 2],
                                 pt_sb[0:MC],
                                 expTI[0:MC, r, 2 * hp:2 * hp + 2])

    # attnv/rowsum PSUM layout, ov tile [C, 2, 512] f32 (2 banks):
    #   attnv head h  -> bank 0, out partitions [32h : 32h+32]
    #   rowsum head h -> bank 1, SAME partitions (ones-lhsT [112, 32]
    #                    replicates the rowsum across the head's block, so
    #                    normalization is lane-aligned with no broadcast)
    # all 8 chains accumulate with start=False onto PE-zeroed banks.
    def attnv_round_a(b, r):
        s = state[b]
        if r == 0:
            ov = ps_ov.tile([C, 2, 512], f32, tag="ov")
            s["ov"] = ov
            rhs392 = expTI[0:MC, 0, 0, :]
            for j in range(2):
                nc.tensor.matmul(ov[:, j, 0:NSL], lhsT=zeros_sb[0:MC, :],
                                 rhs=rhs392, start=True, stop=True)
        else:
            ov = s["ov"]
        for h in range(HEADS):
            nc.tensor.matmul(
                ov[HD * h:HD * (h + 1), 0, 0:NSL],
                lhsT=s["v"][0:MC, r, h, :],
                rhs=s["pp"][0:MC, r, h, :],
                start=False, stop=(r == NR - 1),
                tile_position=(0, HD * h),
                skip_group_check=True)

    def attnv_round_b(b, r):
        s = state[b]
        ov = s["ov"]
        for h in range(HEADS):
            nc.tensor.matmul(
                ov[HD * h:HD * (h + 1), 1, 0:NSL],
                lhsT=ones_sb[0:MC, :],
                rhs=s["pp"][0:MC, r, h, :],
                start=False, stop=(r == NR - 1),
                tile_position=(0, HD * h),
                skip_group_check=True)

    def finish(b):
        """rowsum recip-broadcast -> fused normalize -> proj (psum-free:
        reuses the batch's own ov bank 0 for the projection output)."""
        s = state[b]
        ov = s.pop("ov")
        rb_sb = opool.tile([C, NSL], f32, tag="rb")
        nc.vector.reciprocal_approx_fast(rb_sb[:], ov[:, 1, 0:NSL])
        outT_sb = opool.tile([C, NSL], bf16, tag="outT")
        nc.vector.tensor_mul(outT_sb[:], ov[:, 0, 0:NSL], rb_sb[:])
        ps_ft = ps_sco.tile([C, 2, 512], f32, tag="sco", name="ps_ft")
        nc.tensor.matmul(ps_ft[:, 0, 0:NSL], lhsT=wp_sb[:], rhs=outT_sb[:],
                         start=True, stop=True)
        fin_sb = opool.tile([C, NSL], f32, tag="fin")
        nc.vector.tensor_copy(fin_sb[:], ps_ft[:, 0, 0:NSL])
        nc.sync.dma_start(out=out_d[b], in_=fin_sb[:])
        state.pop(b)

    # ---- schedule ----
    # attnv for global round g is emitted at slot g+LAG so the PE's in-order
    # matmul queue never blocks next-round scores behind the exp/mult chain.
    # Emitted BEFORE the slot's score matmuls: while those wait for the exp
    # ping-pong to free their PSUM buffer, the PE chews the ready attnv work.
    LAG = 2

    def attnv_slot_a(g):
        if g < 0:
            return
        bb, rr = divmod(g, NR)
        if rr == 0 and bb >= 1:
            finish(bb - 1)
        if bb < B:
            attnv_round_a(bb, rr)

    def attnv_slot_b(g):
        if g < 0:
            return
        bb, rr = divmod(g, NR)
        if bb < B:
            attnv_round_b(bb, rr)

    prep_load(0)
    load_expTI()
    prep_q(0)
    prep_k(0, 0)
    prep_k(0, 1)
    prep_v(0, 0)
    prep_v(0, 1)
    prep_load(1)
    pp0 = ppool.tile([C, NR, HEADS, NSL], bf16, tag="pp")
    state[0]["pp"] = pp0
    for b in range(B):
        for r in range(NR):
            # round 0 of batches 1.. was pre-emitted at the previous
            # batch's r6 slot (ahead of the v-prep borrow) so the exp
            # pipeline never drains across the batch boundary
            if r != 0 or b == 0:
                score_round(b, r)
            attnv_slot_a(NR * b + r - LAG)
            attnv_slot_b(NR * b + r - LAG)
            if b + 1 < B:
                if r == 2:
                    prep_q(b + 1)
                elif r == 3:
                    prep_k(b + 1, 0)
                elif r == 4:
                    prep_k(b + 1, 1)
                elif r == 5:
                    prep_v(b + 1, 0)
                elif r == 6:
                    pp_sb = ppool.tile([C, NR, HEADS, NSL], bf16, tag="pp")
                    state[b + 1]["pp"] = pp_sb
                    score_round(b + 1, 0)
                    prep_v(b + 1, 1)
                    if b + 2 < B:
                        prep_load(b + 2)
    for g in range(B * NR - LAG, B * NR + 1):
        attnv_slot_a(g)
        attnv_slot_b(g)


def _get_compiled():
    global _COMPILED
    if _COMPILED is None:
        _COMPILED = _build()
    return _COMPILED


def make_in_map(prep, j):
    return {
        "xTn": np.ascontiguousarray(prep["xT"][:, :, j * NSL:(j + 1) * NSL]),
        "xkT": prep["xkT"],
        "expRT": prep["expRT"][j],
        "Wq": prep["Wq"], "bq": prep["bq"],
        "Wk": prep["Wk"], "Wv": prep["Wv"], "Wp": prep["Wp"],
    }


def kernel(x, relative_pos, Wq, bq, Wk, bk, Wv, bv, conv_w, conv_b,
           bn_gamma, bn_beta, bn_mean, bn_var, Wp, bp, H=56, W=56,
           _trace=False):
    from concourse.bass_utils import run_bass_kernel_spmd

    prep = _host_prep(x, relative_pos, Wq, bq, Wk, bk, Wv, bv, conv_w,
                      conv_b, bn_gamma, bn_beta, bn_mean, bn_var, Wp, bp)
    nc = _get_compiled()

    in_maps = [make_in_map(prep, j) for j in range(NCORES)]

    res = run_bass_kernel_spmd(nc, in_maps, core_ids=list(range(NCORES)),
                               trace=_trace)

    out = np.empty((B, N, C), np.float32)
    for j in range(NCORES):
        out[:, j * NSL:(j + 1) * NSL, :] = \
            res.results[j]["out"].transpose(0, 2, 1)
    out += prep["bp"][None, None, :]
    if _trace:
        kernel._last_result = res
    return out


# revision 48
# speedup vs baseline: 1.0455x; 1.0455x over previous
"""Trainium2 Bass kernel for PVT-style spatial-reduction attention.

Model (see reference):
  q = (x @ Wq + bq) * hd^-0.5                       (B, N, C) -> heads of 32
  x_ = BN(DWConv2x2s2(x)) ; k = x_ @ Wk ; v = x_ @ Wv
  attn = softmax(q k^T + rel_pos) ; out = (attn @ v) @ Wp + bp

Shapes: B=8, N=3136 (56x56), C=128, heads=4, hd=32, Nkv=784 (28x28).

Distribution: each of 8 cores handles a slice of 392 query rows (N/8) for
ALL batches and heads; rel_pos splits exactly 8 ways; no collectives.

Device layout strategy (v2):
  - host folds conv+BN (exact fp32) producing x_^T (B, C, 784) bf16; k-bias
    dropped (softmax-invariant), v-bias folded into the final bias (added on
    host during the gather).
  - kv chunking is 7 rounds x 112 (784 = 7*112 exactly).
  - scores are computed transposed S^T[m, n] per (b, h) in half-rounds of 2
    heads on concurrent 32-row PE tiles -> [C, 2, 512] f32 PSUM (2 banks,
    2 bufs); one ScalarE exp per half-round (FD=784); softmax uses
    exp(S)*exp(R) with exp(rel^T) bf16 precomputed host side; ScalarE runs
    NOTHING but exp.
  - v is produced directly in [m, d] layout (lhsT = x_^T chunk streaming
    Wv), all chunks through a single rotating PSUM bank -> no PE
    transposes at all.
  - attn@v: per round 4 col-packed matmuls (head h -> bank 0, output
    partitions 32h..32h+31) + 4 col-packed ones-lhsT [112, 32] matmuls
    that write each head's row sums REPLICATED across the same partition
    block of bank 1, accumulating f32 over the 7 rounds onto PE-zeroed
    banks.  The attnv output lands directly in outT [d, n] partition
    order and the row sums are lane-aligned with it, so normalization
    is one reciprocal + one multiply per batch, with the projection
    reusing the freed attnv bank.
  - final output is produced transposed (B, C, NSL); the host gather
    untransposes while assembling the full (B, N, C) result and adds the
    folded output bias.
"""

import sys

import numpy as np

if "/opt/trn_rl_repo" not in sys.path:
    sys.path.insert(0, "/opt/trn_rl_repo")

B = 8
N = 3136
C = 128
HEADS = 4
HD = 32
SR = 2
H = W = 56
NKV = 784  # 28*28
NCORES = 8
NSL = N // NCORES  # 392 query rows per core
BN_EPS = 1e-5
SCALE = HD ** -0.5

MC = 112            # kv chunk size
NR = NKV // MC      # 7 rounds

_COMPILED = None  # cached nc across kernel() calls


def _host_prep(x, relative_pos, Wq, bq, Wk, bk, Wv, bv, conv_w, conv_b,
               bn_gamma, bn_beta, bn_mean, bn_var, Wp, bp):
    """Fold conv/BN on host (exact fp32); fold biases; transpose activations."""
    import ml_dtypes
    f32 = np.float32
    bf16 = ml_dtypes.bfloat16
    x = np.asarray(x, f32)
    # xT: (B, C, N) for the q projection (each core slices its own 392 cols)
    xT = np.ascontiguousarray(x.transpose(0, 2, 1)).astype(bf16)

    # conv + BN (exact, running stats)
    inv = (np.asarray(bn_gamma, f32)
           / np.sqrt(np.asarray(bn_var, f32) + BN_EPS))          # [c]
    cw = np.asarray(conv_w, f32).reshape(C, SR, SR)
    x_img = x.transpose(0, 2, 1).reshape(B, C, H, W)
    y = np.zeros((B, C, H // SR, W // SR), f32)
    for di in range(SR):
        for dj in range(SR):
            y += x_img[:, :, di::SR, dj::SR] * cw[None, :, di, dj, None, None]
    y += np.asarray(conv_b, f32)[None, :, None, None]
    y = y * inv[None, :, None, None] \
        + (np.asarray(bn_beta, f32)
           - np.asarray(bn_mean, f32) * inv)[None, :, None, None]
    xkT = np.ascontiguousarray(y.reshape(B, C, NKV)).astype(bf16)  # (B, C, 784)

    # v bias (uniform over kv positions -> exact fold into final bias)
    bp_host = (np.asarray(bp, np.float64)
               + np.asarray(bv, np.float64) @ np.asarray(Wp, np.float64))
    bp_host = bp_host.astype(f32)

    Wq_s = np.ascontiguousarray((np.asarray(Wq, f32) * SCALE)).astype(bf16)
    bq_col = (np.asarray(bq, f32) * SCALE).reshape(C, 1).astype(f32)
    Wk_b = np.ascontiguousarray(np.asarray(Wk, f32)).astype(bf16)
    Wv_b = np.ascontiguousarray(np.asarray(Wv, f32)).astype(bf16)
    Wp_b = np.ascontiguousarray(np.asarray(Wp, f32)).astype(bf16)

    # exp(rel)^T per core: (4, NKV, NSL) bf16
    rel = np.asarray(relative_pos, f32)
    expRT = []
    for j in range(NCORES):
        sl = rel[:, j * NSL:(j + 1) * NSL, :]          # (4, NSL, NKV)
        e = np.exp(sl).transpose(0, 2, 1).astype(bf16)  # (4, NKV, NSL)
        expRT.append(np.ascontiguousarray(e))

    return dict(xT=xT, xkT=xkT, Wq=Wq_s, bq=bq_col, Wk=Wk_b, Wv=Wv_b,
                Wp=Wp_b, bp=bp_host, expRT=expRT)


def _build():
    """Build + compile the SPMD bass program (same NEFF for all 8 cores)."""
    import concourse.bass as bass
    import concourse.tile as tile
    from concourse import bacc, mybir

    f32 = mybir.dt.float32
    f32r = mybir.dt.float32r
    bf16 = mybir.dt.bfloat16

    nc = bacc.Bacc("TRN2", target_bir_lowering=False, debug=False,
                   num_devices=NCORES)

    # ---- DRAM I/O ----
    xTn_d = nc.dram_tensor("xTn", [B, C, NSL], bf16, kind="ExternalInput").ap()
    xkT_d = nc.dram_tensor("xkT", [B, C, NKV], bf16, kind="ExternalInput").ap()
    expRT_d = nc.dram_tensor("expRT", [HEADS, NKV, NSL],
                             bf16, kind="ExternalInput").ap()
    Wq_d = nc.dram_tensor("Wq", [C, C], bf16, kind="ExternalInput").ap()
    bq_d = nc.dram_tensor("bq", [C, 1], f32, kind="ExternalInput").ap()
    Wk_d = nc.dram_tensor("Wk", [C, C], bf16, kind="ExternalInput").ap()
    Wv_d = nc.dram_tensor("Wv", [C, C], bf16, kind="ExternalInput").ap()
    Wp_d = nc.dram_tensor("Wp", [C, C], bf16, kind="ExternalInput").ap()
    out_d = nc.dram_tensor("out", [B, C, NSL], f32, kind="ExternalOutput").ap()

    with tile.TileContext(nc) as tc:
        from contextlib import ExitStack
        with ExitStack() as ctx:
            _emit(ctx, tc, nc, bass, mybir, f32, f32r, bf16,
                  xTn_d, xkT_d, expRT_d, Wq_d, bq_d, Wk_d, Wv_d, Wp_d,
                  out_d)

    nc.compile()
    return nc


def _emit(ctx, tc, nc, bass, mybir, f32, f32r, bf16,
          xTn_d, xkT_d, expRT_d, Wq_d, bq_d, Wk_d, Wv_d, Wp_d,
          out_d):
    AF = mybir.ActivationFunctionType

    singles = ctx.enter_context(tc.tile_pool(name="singles", bufs=1))
    xpool = ctx.enter_context(tc.tile_pool(name="xpool", bufs=3))
    qkv = ctx.enter_context(tc.tile_pool(name="qkv", bufs=3))
    ptpool = ctx.enter_context(tc.tile_pool(name="ptpool", bufs=4))
    ppool = ctx.enter_context(tc.tile_pool(name="ppool", bufs=2))
    opool = ctx.enter_context(tc.tile_pool(name="opool", bufs=3))
    # PSUM (8 banks): sco 3 tiles x 2 banks (also borrowed by the small
    # prep/proj matmuls; 3-deep so score matmuls never head-block the PE
    # queue waiting on the exp ping-pong), ov 1 tile x 2 banks.
    ps_sco = ctx.enter_context(tc.tile_pool(name="ps_sco", bufs=3,
                                            space="PSUM"))
    ps_ov = ctx.enter_context(tc.tile_pool(name="ps_ov", bufs=1, space="PSUM"))

    # ---- constants ----
    wq_sb = singles.tile([C, C], bf16)
    nc.sync.dma_start(out=wq_sb[:], in_=Wq_d)
    bq_sb = singles.tile([C, 1], f32)
    nc.sync.dma_start(out=bq_sb[:], in_=bq_d)
    wk_sb = singles.tile([C, C], bf16)
    nc.sync.dma_start(out=wk_sb[:], in_=Wk_d)
    wv_sb = singles.tile([C, C], bf16)
    nc.sync.dma_start(out=wv_sb[:], in_=Wv_d)
    wp_sb = singles.tile([C, C], bf16)
    nc.sync.dma_start(out=wp_sb[:], in_=Wp_d)
    ones_sb = singles.tile([C, HD], bf16)
    nc.vector.memset(ones_sb[:], 1.0)
    zeros_sb = singles.tile([C, C], bf16)
    nc.vector.memset(zeros_sb[:], 0.0)

    # expRT interleaved: [112, 7 rounds, 4 heads, 392].  The DMAs are
    # emitted in the schedule after the batch-0 input loads so the first
    # scores are not queued behind 2.5MB of rel_pos traffic.
    expTI = singles.tile([C, NR, HEADS, NSL], bf16)

    def load_expTI():
        for h in range(HEADS):
            nc.sync.dma_start(
                out=expTI[0:MC, :, h, :],
                in_=expRT_d[h].rearrange("(j p) i -> p j i", p=MC))

    state = {}

    def prep_load(b):
        s = state.setdefault(b, {})
        xkT_sb = xpool.tile([C, NKV], bf16, tag="xkT")
        s["xkT"] = xkT_sb
        nc.sync.dma_start(out=xkT_sb[:], in_=xkT_d[b])
        xTn_sb = xpool.tile([C, NSL], bf16, tag="xTn")
        s["xTn"] = xTn_sb
        nc.sync.dma_start(out=xTn_sb[:], in_=xTn_d[b])

    def prep_q(b):
        s = state[b]
        ps_q = ps_sco.tile([C, 2, 512], f32, tag="sco")
        nc.tensor.matmul(ps_q[:, 0, 0:NSL], lhsT=wq_sb[:], rhs=s.pop("xTn")[:],
                         start=True, stop=True)
        qT_sb = qkv.tile([C, NSL], bf16, tag="qT")
        s["qT"] = qT_sb
        nc.vector.tensor_scalar_add(qT_sb[:], ps_q[:, 0, 0:NSL], bq_sb[:, 0:1])

    def prep_k(b, half):
        s = state[b]
        if half == 0:
            kT_sb = qkv.tile([C, NKV], bf16, tag="kT")
            s["kT"] = kT_sb
        ps_k = ps_sco.tile([C, 2, 512], f32, tag="sco")
        nc.tensor.matmul(ps_k[:, 0, 0:NSL], lhsT=wk_sb[:],
                         rhs=s["xkT"][:, half * NSL:(half + 1) * NSL],
                         start=True, stop=True)
        nc.vector.tensor_copy(s["kT"][:, half * NSL:(half + 1) * NSL],
                              ps_k[:, 0, 0:NSL])

    def prep_v(b, part):
        """v chunks in [m, d] layout: lhsT = xkT chunk, rhs = Wv."""
        s = state[b]
        if part == 0:
            v_sb = qkv.tile([C, NR, HEADS, HD], bf16, tag="v")
            s["v"] = v_sb
            rr = range(0, 4)
        else:
            rr = range(4, NR)
        ps_v = ps_sco.tile([C, 2, 512], f32, tag="sco")
        pv = ps_v[:].rearrange("p j (a d) -> p (j a) d", d=C)
        for i, r in enumerate(rr):
            nc.tensor.matmul(pv[0:MC, i, :],
                             lhsT=s["xkT"][:, MC * r:MC * (r + 1)],
                             rhs=wv_sb[:], start=True, stop=True)
        nv = len(rr)
        nc.vector.tensor_copy(
            s["v"][0:MC, 4 * part:4 * part + nv]
                .rearrange("p r h d -> p (r h d)"),
            pv[0:MC, 0:nv, :].rearrange("p a d -> p (a d)"))
        if part == 1:
            s.pop("xkT")

    def score_round(b, r):
        """All 4 head score matmuls back-to-back (4-way row-strip
        concurrency across two 2-bank tiles), then exp + expR multiply
        per head pair."""
        s = state[b]
        tiles = (ps_sco.tile([C, 2, 512], f32, tag="sco", name="ps_sA"),
                 ps_sco.tile([C, 2, 512], f32, tag="sco", name="ps_sB"))
        for h in range(HEADS):
            nc.tensor.matmul(
                tiles[h // 2][0:MC, h % 2, 0:NSL],
                lhsT=s["kT"][HD * h:HD * (h + 1), MC * r:MC * (r + 1)],
                rhs=s["qT"][HD * h:HD * (h + 1), :],
                start=True, stop=True,
                tile_position=(HD * h, 0))
        for hp, t in enumerate(tiles):
            pt_sb = ptpool.tile([C, 2, NSL], bf16, tag="pt")
            nc.scalar.activation(pt_sb[0:MC], t[0:MC, :, 0:NSL], AF.Exp)
            nc.vector.tensor_mul(s["pp"][0:MC, r, 2 * hp:2 * hp + 2],
                                 pt_sb[0:MC],
                                 expTI[0:MC, r, 2 * hp:2 * hp + 2])

    # attnv/rowsum PSUM layout, ov tile [C, 2, 512] f32 (2 banks):
    #   attnv head h  -> bank 0, out partitions [32h : 32h+32]
    #   rowsum head h -> bank 1, SAME partitions (ones-lhsT [112, 32]
    #                    replicates the rowsum across the head's block, so
    #                    normalization is lane-aligned with no broadcast)
    # all 8 chains accumulate with start=False onto PE-zeroed banks.
    def attnv_round_a(b, r):
        s = state[b]
        if r == 0:
            ov = ps_ov.tile([C, 2, 512], f32, tag="ov")
            s["ov"] = ov
            rhs392 = expTI[0:MC, 0, 0, :]
            for j in range(2):
                nc.tensor.matmul(ov[:, j, 0:NSL], lhsT=zeros_sb[0:MC, :],
                                 rhs=rhs392, start=True, stop=True)
        else:
            ov = s["ov"]
        for h in range(HEADS):
            nc.tensor.matmul(
                ov[HD * h:HD * (h + 1), 0, 0:NSL],
                lhsT=s["v"][0:MC, r, h, :],
                rhs=s["pp"][0:MC, r, h, :],
                start=False, stop=(r == NR - 1),
                tile_position=(0, HD * h),
                skip_group_check=True)

    def attnv_round_b(b, r):
        s = state[b]
        ov = s["ov"]
        for h in range(HEADS):
            nc.tensor.matmul(
                ov[HD * h:HD * (h + 1), 1, 0:NSL],
                lhsT=ones_sb[0:MC, :],
                rhs=s["pp"][0:MC, r, h, :],
                start=False, stop=(r == NR - 1),
                tile_position=(0, HD * h),
                skip_group_check=True)

    def finish(b):
        """rowsum recip-broadcast -> fused normalize -> proj (psum-free:
        reuses the batch's own ov bank 0 for the projection output)."""
        s = state[b]
        ov = s.pop("ov")
        rb_sb = opool.tile([C, NSL], f32, tag="rb")
        nc.vector.reciprocal_approx_fast(rb_sb[:], ov[:, 1, 0:NSL])
        outT_sb = opool.tile([C, NSL], bf16, tag="outT")
        nc.vector.tensor_mul(outT_sb[:], ov[:, 0, 0:NSL], rb_sb[:])
        ps_ft = ps_sco.tile([C, 2, 512], f32, tag="sco", name="ps_ft")
        nc.tensor.matmul(ps_ft[:, 0, 0:NSL], lhsT=wp_sb[:], rhs=outT_sb[:],
                         start=True, stop=True)
        fin_sb = opool.tile([C, NSL], f32, tag="fin")
        nc.vector.tensor_copy(fin_sb[:], ps_ft[:, 0, 0:NSL])
        nc.sync.dma_start(out=out_d[b], in_=fin_sb[:])
        state.pop(b)

    # ---- schedule ----
    # attnv for global round g is emitted at slot g+LAG so the PE's in-order
    # matmul queue never blocks next-round scores behind the exp/mult chain.
    # Emitted BEFORE the slot's score matmuls: while those wait for the exp
    # ping-pong to free their PSUM buffer, the PE chews the ready attnv work.
    LAG = 2

    def attnv_slot_a(g):
        if g < 0:
            return
        bb, rr = divmod(g, NR)
        if rr == 0 and bb >= 1:
            finish(bb - 1)
        if bb < B:
            attnv_round_a(bb, rr)

    def attnv_slot_b(g):
        if g < 0:
            return
        bb, rr = divmod(g, NR)
        if bb < B:
            attnv_round_b(bb, rr)

    prep_load(0)
    load_expTI()
    prep_q(0)
    prep_k(0, 0)
    prep_k(0, 1)
    prep_v(0, 0)
    prep_v(0, 1)
    prep_load(1)
    pp0 = ppool.tile([C, NR, HEADS, NSL], bf16, tag="pp")
    state[0]["pp"] = pp0
    for b in range(B):
        for r in range(NR):
            # round 0 of batches 1.. was pre-emitted at the previous
            # batch's r6 slot (ahead of the v-prep borrow) so the exp
            # pipeline never drains across the batch boundary
            if r != 0 or b == 0:
                score_round(b, r)
            attnv_slot_a(NR * b + r - LAG)
            attnv_slot_b(NR * b + r - LAG)
            if b + 1 < B:
                if r == 2:
                    prep_q(b + 1)
                elif r == 3:
                    prep_k(b + 1, 0)
                elif r == 4:
                    prep_k(b + 1, 1)
                elif r == 5:
                    prep_v(b + 1, 0)
                elif r == 6:
                    pp_sb = ppool.tile([C, NR, HEADS, NSL], bf16, tag="pp")
                    state[b + 1]["pp"] = pp_sb
                    score_round(b + 1, 0)
                    prep_v(b + 1, 1)
                    if b + 2 < B:
                        prep_load(b + 2)
    for g in range(B * NR - LAG, B * NR + 1):
        attnv_slot_a(g)
        attnv_slot_b(g)


def _get_compiled():
    global _COMPILED
    if _COMPILED is None:
        _COMPILED = _build()
    return _COMPILED


def make_in_map(prep, j):
    return {
        "xTn": np.ascontiguousarray(prep["xT"][:, :, j * NSL:(j + 1) * NSL]),
        "xkT": prep["xkT"],
        "expRT": prep["expRT"][j],
        "Wq": prep["Wq"], "bq": prep["bq"],
        "Wk": prep["Wk"], "Wv": prep["Wv"], "Wp": prep["Wp"],
    }


def kernel(x, relative_pos, Wq, bq, Wk, bk, Wv, bv, conv_w, conv_b,
           bn_gamma, bn_beta, bn_mean, bn_var, Wp, bp, H=56, W=56,
           _trace=False):
    from concourse.bass_utils import run_bass_kernel_spmd

    prep = _host_prep(x, relative_pos, Wq, bq, Wk, bk, Wv, bv, conv_w,
                      conv_b, bn_gamma, bn_beta, bn_mean, bn_var, Wp, bp)
    nc = _get_compiled()

    in_maps = [make_in_map(prep, j) for j in range(NCORES)]

    res = run_bass_kernel_spmd(nc, in_maps, core_ids=list(range(NCORES)),
                               trace=_trace)

    out = np.empty((B, N, C), np.float32)
    for j in range(NCORES):
        out[:, j * NSL:(j + 1) * NSL, :] = \
            res.results[j]["out"].transpose(0, 2, 1)
    out += prep["bp"][None, None, :]
    if _trace:
        kernel._last_result = res
    return out
